# revision 1
# baseline (speedup 1.0000x reference)
"""NeuralMemory fast-weight recurrence on 8 Trainium2 NeuronCores.

Sharding: 8-way tensor-parallel over the memory dim M=2048 (m_s=256/core).
Per chunk: MLP forward, analytic MSE backward, gated fast-weight update,
re-forward. Cross-core: one bf16 AllReduce per chunk of the partial layer-2
activation (pred); the final `out` is returned as per-core partial sums that
the host adds (sum-sharded gather).

Numerics: bf16 matmul operands, fp32 PSUM accumulation. Weights are kept in
"Q-space" (divided by the running forget product c_j) so each update is a
single fused scaled-add (DVE scalar_tensor_tensor reading the gradient PSUM);
the forward applies c via ScalarEngine fused scale. Transposed layouts
(xT, hT, dpredT) are produced with PE-transposes (identity matmul) +
DVE/ACT PSUM evacuation; gW1's second layout comes from a swapped matmul.
"""
import numpy as np
import concourse.bacc as bacc
import concourse.mybir as mybir
import concourse.tile as tile
from concourse.bass_utils import run_bass_kernel_spmd

BF = mybir.dt.bfloat16
F32 = mybir.dt.float32
AF = mybir.ActivationFunctionType
ALU = mybir.AluOpType

NCORES = 8
B, L, D, M = 2, 2048, 2048, 2048
C = 128                 # reference CHUNK
NCH = L // C            # 16 chunks
T = B * C               # 256 tokens per chunk
MS = M // NCORES        # 256 per-core memory slice
KD = D // 128           # 16 tiles over D
KT = T // 128           # 2 tiles over tokens
KM = MS // 128          # 2 tiles over m_s
NN = D // 512           # 4 N-chunks of 512 over D
LR_MEMORY = 0.01


def build(no_ar=False):
    nc = bacc.Bacc("TRN2", target_bir_lowering=False, num_devices=NCORES)
    x = nc.dram_tensor("x", [B, L, D], F32, kind="ExternalInput")
    w0t_in = nc.dram_tensor("w0t", [D, MS], F32, kind="ExternalInput")
    w1t_in = nc.dram_tensor("w1t", [MS, D], F32, kind="ExternalInput")
    w1n_in = nc.dram_tensor("w1n", [D, MS], F32, kind="ExternalInput")
    lrfg_in = nc.dram_tensor("lrfg", [D, 2], F32, kind="ExternalInput")
    lrb_in = nc.dram_tensor("lrb", [1, 1], F32, kind="ExternalInput")
    fgb_in = nc.dram_tensor("fgb", [1, 1], F32, kind="ExternalInput")
    b0_in = nc.dram_tensor("b0", [1, MS], F32, kind="ExternalInput")
    b1d8_in = nc.dram_tensor("b1d8", [1, D], F32, kind="ExternalInput")
    ident_in = nc.dram_tensor("ident", [128, 128], F32, kind="ExternalInput")
    outp = nc.dram_tensor("outp", [B, L, D], F32, kind="ExternalOutput")

    with tile.TileContext(nc) as tc:
        with (
            tc.tile_pool(name="wp", bufs=1) as wp,          # persistent weights/consts
            tc.tile_pool(name="xp", bufs=2) as xp,          # per-chunk x streams (prefetched)
            tc.tile_pool(name="ap", bufs=2) as ap,          # loop-carried activations
            tc.tile_pool(name="tp", bufs=1) as tp,          # within-iteration temporaries
            tc.tile_pool(name="sp", bufs=2) as spool,       # tiny scalar tiles
            tc.tile_pool(name="psA", bufs=2, space="PSUM") as psA,   # [128,512]
            tc.tile_pool(name="psB", bufs=2, space="PSUM") as psB,   # [128,256]
            tc.tile_pool(name="psD", bufs=2, space="PSUM") as psD,   # small rows
            tc.tile_pool(name="psT", bufs=2, space="PSUM") as psT,   # transpose blocks
            tc.tile_pool(name="dr", bufs=2, space="DRAM") as dr,
        ):
            # ---------------- persistent weights (bf16, Q-space) ----------------
            q0t = wp.tile([128, KD * MS], BF, name="q0t")      # W0^T: d-tile i at cols i*MS
            q1t = wp.tile([128, KM * D], BF, name="q1t")       # W1^T: m-tile k at cols k*D
            q1n = wp.tile([128, KD * MS], BF, name="q1n")      # W1:   d-tile i at cols i*MS
            bk0 = wp.tile([128, MS], BF, name="bk0")           # row0 = qb0
            bk1 = wp.tile([128, D], BF, name="bk1")            # row0 = qb1/8
            lrfg = wp.tile([128, KD * 2], BF, name="lrfg")     # d-tile i at cols 2i..2i+1
            ones_row = wp.tile([128, 128], BF, name="ones_row")  # row0 = 1, rest 0
            ones_col = wp.tile([128, 1], BF, name="ones_col")    # all ones
            ident = wp.tile([128, 128], BF, name="ident")        # identity (PE transpose)
            lrb_sb = wp.tile([1, 1], F32, name="lrb_sb")
            fgb_sb = wp.tile([1, 1], F32, name="fgb_sb")

            w0t3 = w0t_in.rearrange("(i p) m -> i p m", p=128)
            w1t3 = w1t_in.rearrange("(k p) d -> k p d", p=128)
            w1n3 = w1n_in.rearrange("(i p) m -> i p m", p=128)
            lrfg3 = lrfg_in.rearrange("(i p) g -> i p g", p=128)
            for i in range(KD):
                nc.gpsimd.dma_start(q0t[:, i * MS:(i + 1) * MS], w0t3[i])
                nc.gpsimd.dma_start(q1n[:, i * MS:(i + 1) * MS], w1n3[i])
                nc.gpsimd.dma_start(lrfg[:, 2 * i:2 * i + 2], lrfg3[i])
            for k in range(KM):
                nc.gpsimd.dma_start(q1t[:, k * D:(k + 1) * D], w1t3[k])
            nc.gpsimd.memset(bk0[:], 0.0)
            nc.gpsimd.memset(bk1[:], 0.0)
            nc.gpsimd.dma_start(bk0[0:1, :], b0_in[:])
            nc.gpsimd.dma_start(bk1[0:1, :], b1d8_in[:])
            nc.gpsimd.memset(ones_row[:], 0.0)
            nc.vector.memset(ones_row[0:1, :], 1.0)
            nc.vector.memset(ones_col[:], 1.0)
            nc.gpsimd.dma_start(ident[:], ident_in[:])
            nc.sync.dma_start(lrb_sb[:], lrb_in[:])
            nc.sync.dma_start(fgb_sb[:], fgb_in[:])

            # running forget product c (scalar state), starts at 1
            c11 = spool.tile([1, 1], F32, name="c11")
            c_bc = spool.tile([128, 1], F32, name="c_bc")
            nc.vector.memset(c11[:], 1.0)
            nc.gpsimd.partition_broadcast(c_bc[:], c11[:])

            # ---------------- helpers ----------------
            def pe_transpose(dst, dst_col, src, src_col, j, who, ei):
                """dst[:, dst_col:+128] = src[:, src_col:+128].T via PE."""
                pt = psT.tile([128, 128], BF, name=f"tp{who}_{j}_{dst_col}", tag="psT")
                nc.tensor.transpose(pt[:], src[:, src_col:src_col + 128], ident[:])
                if ei == 0:
                    nc.vector.tensor_copy(dst[:, dst_col:dst_col + 128], pt[:])
                else:
                    nc.scalar.copy(dst[:, dst_col:dst_col + 128], pt[:])

            def load_chunk(j):
                xb = xp.tile([128, KT * D], BF, name=f"xb{j}", tag="xb")
                for t in range(KT):
                    nc.gpsimd.dma_start(xb[:, t * D:(t + 1) * D], x[t, j * C:(j + 1) * C, :])
                return xb

            def transpose_x(xb, j):
                xT = xp.tile([128, KD * T], BF, name=f"xT{j}", tag="xT")
                for t in range(KT):
                    for i in range(KD):
                        pe_transpose(xT, i * T + t * 128, xb, t * D + i * 128, j, "x",
                                     (t * KD + i) % 2)
                return xT

            def mm1(xT, cb, want_dsilu, j, pfx, pool):
                h = pool.tile([128, KT * MS], BF, name=f"h{pfx}_{j}", tag=f"h{pfx}")
                hp = None
                if want_dsilu:
                    hp = pool.tile([128, KT * MS], BF, name=f"hp_{j}", tag="hp")
                pts = []
                for t in range(KT):
                    pt = psB.tile([128, MS], F32, name=f"psh{pfx}_{j}_{t}", tag="psB")
                    for i in range(KD):
                        nc.tensor.matmul(pt[:], xT[:, i * T + t * 128:i * T + (t + 1) * 128],
                                         q0t[:, i * MS:(i + 1) * MS],
                                         start=(i == 0), stop=False)
                    nc.tensor.matmul(pt[:], ones_row[:], bk0[:], start=False, stop=True)
                    pts.append(pt)
                for t in range(KT):  # group by ACT function to limit table reloads
                    nc.scalar.activation(h[:, t * MS:(t + 1) * MS], pts[t][:], AF.Silu,
                                         scale=cb[:, 0:1])
                if want_dsilu:
                    for t in range(KT):
                        nc.scalar.activation(hp[:, t * MS:(t + 1) * MS], pts[t][:],
                                             AF.Derivative_silu, scale=cb[:, 0:1])
                return h, hp

            def transpose_h(h, j, pfx, pool):
                hT = pool.tile([128, KM * T], BF, name=f"hT{pfx}_{j}", tag=f"hT{pfx}")
                for t in range(KT):
                    for k in range(KM):
                        pe_transpose(hT, k * T + t * 128, h, t * MS + k * 128, j,
                                     f"h{pfx}", (t + k) % 2)
                return hT

            def mm2(hT, cb, out_dtype, j, pfx, pool):
                o = pool.tile([128, KT * D], out_dtype, name=f"o{pfx}_{j}", tag=f"o{pfx}")
                for t in range(KT):
                    for n in range(NN):
                        pt = psA.tile([128, 512], F32, name=f"psp{pfx}_{j}_{t}_{n}", tag="psA")
                        for k in range(KM):
                            nc.tensor.matmul(pt[:], hT[:, k * T + t * 128:k * T + (t + 1) * 128],
                                             q1t[:, k * D + n * 512:k * D + (n + 1) * 512],
                                             start=(k == 0), stop=False)
                        nc.tensor.matmul(pt[:], ones_row[:], bk1[:, n * 512:(n + 1) * 512],
                                         start=False, stop=True)
                        nc.scalar.activation(o[:, t * D + n * 512:t * D + (n + 1) * 512], pt[:],
                                             AF.Copy, scale=cb[:, 0:1])
                return o

            def issue_ar(predp, j):
                arin = dr.tile([T, D], BF, name=f"arin{j}", tag="arin")
                for t in range(KT):
                    nc.gpsimd.dma_start(arin[t * 128:(t + 1) * 128, :],
                                        predp[:, t * D:(t + 1) * D])
                if no_ar:
                    return arin
                arout = dr.tile([T, D], BF, name=f"arout{j}", tag="arout",
                                addr_space="Shared")
                nc.gpsimd.collective_compute(
                    "AllReduce", ALU.add, replica_groups=[list(range(NCORES))],
                    ins=[arin.opt()], outs=[arout.opt()])
                return arout

            # ---------------- prologue: chunk 0 forward under P_0 ----------------
            xb_c = load_chunk(0)
            xT_c = transpose_x(xb_c, 0)
            h1_c, hp1_c = mm1(xT_c, c_bc, True, 0, "1", ap)
            h1T_c = transpose_h(h1_c, 0, "1", ap)
            predp = mm2(h1T_c, c_bc, BF, 0, "p", ap)
            ar_cur = issue_ar(predp, 0)

            # ---------------- main loop ----------------
            for j in range(NCH):
                last = (j == NCH - 1)

                # gates from chunk j (independent of the AllReduce)
                g2a = psD.tile([1, T], F32, name=f"g2a_{j}", tag="psD")
                g2b = psD.tile([1, T], F32, name=f"g2b_{j}", tag="psD")
                for i in range(KD):
                    nc.tensor.matmul(g2a[:], lrfg[:, 2 * i:2 * i + 1], xT_c[:, i * T:(i + 1) * T],
                                     start=(i == 0), stop=(i == KD - 1))
                for i in range(KD):
                    nc.tensor.matmul(g2b[:], lrfg[:, 2 * i + 1:2 * i + 2], xT_c[:, i * T:(i + 1) * T],
                                     start=(i == 0), stop=(i == KD - 1))
                sigl = spool.tile([1, T], F32, name=f"sigl{j}", tag="sigl")
                lsum = spool.tile([1, 1], F32, name=f"lsum{j}", tag="lsum")
                nc.scalar.activation(sigl[:], g2a[:], AF.Sigmoid, bias=lrb_sb[0:1, 0:1],
                                     accum_out=lsum[:])
                fparts = spool.tile([1, 2], F32, name=f"fparts{j}", tag="fparts")
                for b in range(B):
                    r = spool.tile([1, 1], F32, name=f"zfr{j}_{b}", tag=f"zfr{b}")
                    nc.vector.tensor_reduce(r[:], g2b[0:1, b * C:(b + 1) * C],
                                            mybir.AxisListType.X, ALU.add)
                    nc.scalar.activation(fparts[:, b:b + 1], r[:], AF.Sigmoid,
                                         bias=fgb_sb[0:1, 0:1], scale=1.0 / C)
                f11 = spool.tile([1, 1], F32, name=f"f11_{j}", tag="f11")
                nc.vector.tensor_reduce(f11[:], fparts[:], mybir.AxisListType.X, ALU.add)
                nc.vector.tensor_scalar_mul(f11[:], f11[:], 0.5)

                # scalars: c'=c*f ; s1 = LR*2/(N*T)*lsum/c' ; s0 = s1*c ; negated
                cn11 = spool.tile([1, 1], F32, name=f"cn{j}", tag="cn11")
                nc.vector.tensor_tensor(cn11[:], c11[:], f11[:], ALU.mult)
                rcn = spool.tile([1, 1], F32, name=f"rcn{j}", tag="rcn")
                nc.vector.reciprocal(rcn[:], cn11[:])
                negs = spool.tile([1, 1], F32, name=f"negs{j}", tag="negs")
                nc.vector.tensor_tensor(negs[:], lsum[:], rcn[:], ALU.mult)
                nc.vector.tensor_scalar_mul(negs[:], negs[:],
                                            float(-LR_MEMORY * 2.0 / (T * D) / T))
                negs8 = spool.tile([1, 1], F32, name=f"negs8_{j}", tag="negs8")
                nc.vector.tensor_scalar_mul(negs8[:], negs[:], 1.0 / 8.0)
                negs0 = spool.tile([1, 1], F32, name=f"negs0_{j}", tag="negs0")
                nc.vector.tensor_tensor(negs0[:], negs[:], c11[:], ALU.mult)
                negs_bc = spool.tile([128, 1], F32, name=f"negsbc{j}", tag="negs_bc")
                nc.gpsimd.partition_broadcast(negs_bc[:], negs[:])
                negs0_bc = spool.tile([128, 1], F32, name=f"negs0bc{j}", tag="negs0_bc")
                nc.gpsimd.partition_broadcast(negs0_bc[:], negs0[:])
                cn_bc = spool.tile([128, 1], F32, name=f"cnbc{j}", tag="cn_bc")
                nc.gpsimd.partition_broadcast(cn_bc[:], cn11[:])

                # prefetch next chunk (DMA + PE transposes)
                if not last:
                    xb_n = load_chunk(j + 1)
                    xT_n = transpose_x(xb_n, j + 1)

                # AllReduce result -> SBUF ; dpred = pred_full - x (raw)
                pred_full = tp.tile([128, KT * D], BF, name=f"pf{j}", tag="pf")
                for t in range(KT):
                    nc.sync.dma_start(pred_full[:, t * D:(t + 1) * D],
                                      ar_cur[t * 128:(t + 1) * 128, :])
                dpred = tp.tile([128, KT * D], BF, name=f"dp{j}", tag="dp")
                for t in range(KT):
                    nc.vector.tensor_sub(dpred[:, t * D:(t + 1) * D],
                                         pred_full[:, t * D:(t + 1) * D],
                                         xb_c[:, t * D:(t + 1) * D])
                dpredT = tp.tile([128, KD * T], BF, name=f"dpT{j}", tag="dpT")
                for t in range(KT):
                    for i in range(KD):
                        pe_transpose(dpredT, i * T + t * 128, dpred, t * D + i * 128, j,
                                     "dp", (t * KD + i) % 2)

                # dh = dpredT(lhsT) x q1n_OLD -> [T x m_s] ; dh_pre = dh * hp'
                dhp = tp.tile([128, KT * MS], BF, name=f"dhp{j}", tag="dhp")
                for t in range(KT):
                    pt = psB.tile([128, MS], F32, name=f"psdh{j}_{t}", tag="psB")
                    for i in range(KD):
                        nc.tensor.matmul(pt[:], dpredT[:, i * T + t * 128:i * T + (t + 1) * 128],
                                         q1n[:, i * MS:(i + 1) * MS],
                                         start=(i == 0), stop=(i == KD - 1))
                    nc.vector.tensor_tensor(dhp[:, t * MS:(t + 1) * MS], pt[:],
                                            hp1_c[:, t * MS:(t + 1) * MS], ALU.mult)

                # gW1t = h1(lhsT) x dpred ; fused update of q1t (x -s1)   [after dh]
                for k in range(KM):
                    for n in range(NN):
                        pt = psA.tile([128, 512], F32, name=f"psg1_{j}_{k}_{n}", tag="psA")
                        for t in range(KT):
                            nc.tensor.matmul(pt[:],
                                             h1_c[:, t * MS + k * 128:t * MS + (k + 1) * 128],
                                             dpred[:, t * D + n * 512:t * D + (n + 1) * 512],
                                             start=(t == 0), stop=(t == KT - 1))
                        sl = slice(k * D + n * 512, k * D + (n + 1) * 512)
                        nc.vector.scalar_tensor_tensor(q1t[:, sl], pt[:], negs_bc[:, 0:1],
                                                       q1t[:, sl], ALU.mult, ALU.add)
                # gW1n = dpred(lhsT) x h1 ; fused update of q1n (x -s1)
                for i in range(KD):
                    pt = psB.tile([128, MS], F32, name=f"psg1n_{j}_{i}", tag="psB")
                    for t in range(KT):
                        nc.tensor.matmul(pt[:], dpred[:, t * D + i * 128:t * D + (i + 1) * 128],
                                         h1_c[:, t * MS:(t + 1) * MS],
                                         start=(t == 0), stop=(t == KT - 1))
                    sl = slice(i * MS, (i + 1) * MS)
                    nc.vector.scalar_tensor_tensor(q1n[:, sl], pt[:], negs_bc[:, 0:1],
                                                   q1n[:, sl], ALU.mult, ALU.add)
                # gb1 -> bk1 row0 (scale -s1/8), chunked [1,512]
                for n in range(NN):
                    gb1p = psD.tile([1, 512], F32, name=f"gb1_{j}_{n}", tag="psD")
                    for t in range(KT):
                        nc.tensor.matmul(gb1p[:], ones_col[:],
                                         dpred[:, t * D + n * 512:t * D + (n + 1) * 512],
                                         start=(t == 0), stop=(t == KT - 1))
                    nc.vector.scalar_tensor_tensor(bk1[0:1, n * 512:(n + 1) * 512], gb1p[:],
                                                   negs8[0:1, 0:1],
                                                   bk1[0:1, n * 512:(n + 1) * 512],
                                                   ALU.mult, ALU.add)

                # gW0t = x(lhsT) x dh_pre -> [D x m_s] ; fused update q0t (x -s0)
                for i in range(KD):
                    pt = psB.tile([128, MS], F32, name=f"psg0_{j}_{i}", tag="psB")
                    for t in range(KT):
                        nc.tensor.matmul(pt[:], xb_c[:, t * D + i * 128:t * D + (i + 1) * 128],
                                         dhp[:, t * MS:(t + 1) * MS],
                                         start=(t == 0), stop=(t == KT - 1))
                    sl = slice(i * MS, (i + 1) * MS)
                    nc.vector.scalar_tensor_tensor(q0t[:, sl], pt[:], negs0_bc[:, 0:1],
                                                   q0t[:, sl], ALU.mult, ALU.add)
                # gb0 -> bk0 row0 (scale -s0)
                gb0p = psD.tile([1, MS], F32, name=f"gb0_{j}", tag="psD")
                for t in range(KT):
                    nc.tensor.matmul(gb0p[:], ones_col[:], dhp[:, t * MS:(t + 1) * MS],
                                     start=(t == 0), stop=(t == KT - 1))
                nc.vector.scalar_tensor_tensor(bk0[0:1, :], gb0p[:], negs0[0:1, 0:1],
                                               bk0[0:1, :], ALU.mult, ALU.add)

                # ---- forward under P_{j+1}: out_j (f32 partial) and pred_{j+1} ----
                h2, _ = mm1(xT_c, cn_bc, False, j, "2", tp)
                h2T = transpose_h(h2, j, "2", tp)
                outsb = mm2(h2T, cn_bc, F32, j, "o", tp)
                for t in range(KT):
                    nc.sync.dma_start(outp[t, j * C:(j + 1) * C, :],
                                      outsb[:, t * D:(t + 1) * D])
                if not last:
                    h1_n, hp1_n = mm1(xT_n, cn_bc, True, j + 1, "1", ap)
                    h1T_n = transpose_h(h1_n, j + 1, "1", ap)
                    predp = mm2(h1T_n, cn_bc, BF, j + 1, "p", ap)
                    ar_cur = issue_ar(predp, j + 1)
                    xb_c, xT_c = xb_n, xT_n
                    h1_c, hp1_c, h1T_c = h1_n, hp1_n, h1T_n
                c11, c_bc = cn11, cn_bc
    nc.compile()
    return nc


_NC_CACHE = None


def _get_nc():
    global _NC_CACHE
    if _NC_CACHE is None:
        _NC_CACHE = build()
    return _NC_CACHE


def make_in_maps(x, W0, b0, W1, b1, lr_w, lr_b, fg_w, fg_b):
    x = np.ascontiguousarray(np.asarray(x, np.float32))
    W0 = np.asarray(W0, np.float32)
    W1 = np.asarray(W1, np.float32)
    lrfg = np.ascontiguousarray(
        np.stack([np.asarray(lr_w, np.float32)[0], np.asarray(fg_w, np.float32)[0]], axis=1))
    ident = np.eye(128, dtype=np.float32)
    in_maps = []
    for s in range(NCORES):
        sl = slice(s * MS, (s + 1) * MS)
        in_maps.append({
            "x": x,
            "w0t": np.ascontiguousarray(W0[sl, :].T),
            "w1t": np.ascontiguousarray(W1[:, sl].T),
            "w1n": np.ascontiguousarray(W1[:, sl]),
            "lrfg": lrfg,
            "lrb": np.asarray(lr_b, np.float32).reshape(1, 1),
            "fgb": np.asarray(fg_b, np.float32).reshape(1, 1),
            "b0": np.ascontiguousarray(np.asarray(b0, np.float32)[sl].reshape(1, MS)),
            "b1d8": np.ascontiguousarray((np.asarray(b1, np.float32) / 8.0).reshape(1, D)),
            "ident": ident,
        })
    return in_maps


def run(inputs, **kw):
    nc = _get_nc()
    in_maps = make_in_maps(**inputs)
    res = run_bass_kernel_spmd(nc, in_maps, core_ids=list(range(NCORES)), **kw)
    out = np.zeros((B, L, D), np.float32)
    for r in res.results:
        out += r["outp"]
    return out, res


def kernel(**inputs) -> np.ndarray:
    out, _ = run(inputs)
    return out

